# revision 4
# baseline (speedup 1.0000x reference)
"""Trainium2 Bass kernel: batched gather + per-element (4x4)@(4x3) matmul.

out[e,i,c] = sum_j inv_matrices[e,i,j] * convection_vectors[mesh_elements[e,j], c]

Sharding: elements across 8 cores (250k each, padded to 250880 = 128*196*10);
the 400k-node table is replicated to every core. Per core, elements are
processed in 10 chunks of 128x196: bulk-DMA indices+matrices in, indirect-DMA
gather of 12B table rows, DVE multiply/add tree, bulk-DMA out.
"""
import os
import sys

sys.path.insert(0, "/opt/trn_rl_repo")
os.environ.setdefault("JAX_PLATFORMS", "cpu")

import numpy as np

from concourse import bass, bacc, mybir, tile
from concourse.bass_utils import run_bass_kernel_spmd

F32 = mybir.dt.float32
I32 = mybir.dt.int32
P = 128

N_NODES = 400000
N_ELEMENTS = 2000000
N_CORES = 8
E_CORE = N_ELEMENTS // N_CORES  # 250000
M = 196
N_CHUNKS = 10
E_PAD = P * M * N_CHUNKS  # 250880

_CACHE = {}
LAST_RESULTS = None


def _ensure_ntff_hook():
    """Register the axon NTFF profile hook if the image's antenv lacks it."""
    import types

    try:
        from antenv import axon_hooks  # noqa: F401

        return
    except ImportError:
        pass
    try:
        import antenv
        from trn_agent_boot.trn_boot import _ntff_profile_via_ctypes

        hook = _ntff_profile_via_ctypes("/opt/axon/libaxon_pjrt.so")
        mod = types.ModuleType("antenv.axon_hooks")
        mod._hook = hook
        mod.get_axon_ntff_profile_hook = lambda: mod._hook
        mod.set_axon_ntff_profile_hook = lambda h: setattr(mod, "_hook", h)
        sys.modules["antenv.axon_hooks"] = mod
        antenv.axon_hooks = mod
    except Exception:
        pass


def _build_nc(n_table=N_NODES, m=M, n_chunks=N_CHUNKS, io_bufs=2, work_bufs=1):
    e_pad = P * m * n_chunks
    nc = bacc.Bacc("TRN2", target_bir_lowering=False, debug=False)
    table = nc.declare_dram_parameter("convection_vectors", [n_table, 3], F32, isOutput=False)
    mesh = nc.declare_dram_parameter("mesh_elements", [e_pad, 4], I32, isOutput=False)
    inv = nc.declare_dram_parameter("inv_matrices", [e_pad, 4, 4], F32, isOutput=False)
    out = nc.declare_dram_parameter("out", [e_pad, 4, 3], F32, isOutput=True)

    with tile.TileContext(nc) as tc:
        with (
            tc.tile_pool(name="io", bufs=io_bufs) as io_pool,
            tc.tile_pool(name="work", bufs=work_bufs) as work_pool,
        ):
            for ci in range(n_chunks):
                e0 = ci * P * m
                e1 = e0 + P * m
                idx_t = io_pool.tile([P, 4 * m], I32, tag="idx")
                inv_t = io_pool.tile([P, 16 * m], F32, tag="inv")
                g_t = io_pool.tile([P, 12 * m], F32, tag="g")
                out_t = io_pool.tile([P, 12 * m], F32, tag="out")

                nc.sync.dma_start(
                    out=idx_t[:, :],
                    in_=mesh[e0:e1, :].rearrange("(p m) j -> p (m j)", p=P),
                )
                nc.scalar.dma_start(
                    out=inv_t[:, :],
                    in_=inv[e0:e1, :, :].rearrange("(p m) i j -> p (m i j)", p=P),
                )
                # HW contract: one index per partition per indirect DMA.
                # Gather k serves ref-slot k (= element m, vertex j, k=m*4+j)
                # for all 128 partitions at once.
                for k in range(4 * m):
                    nc.gpsimd.indirect_dma_start(
                        out=g_t[:, 3 * k : 3 * k + 3],
                        out_offset=None,
                        in_=table[:, :],
                        in_offset=bass.IndirectOffsetOnAxis(ap=idx_t[:, k : k + 1], axis=0),
                    )

                inv4 = inv_t[:, :].rearrange("p (m i j) -> p m i j", i=4, j=4)
                g4 = g_t[:, :].rearrange("p (m j c) -> p m j c", j=4, c=3)
                p_tiles = []
                for j in range(4):
                    pj = work_pool.tile([P, 12 * m], F32, tag=f"p{j}")
                    a = inv4[:, :, :, j]  # [P, m, 4] strides (16, 4)
                    in0 = bass.AP(a.tensor, a.offset, list(a.ap) + [[0, 3]])
                    gj = g4[:, :, j, :]  # [P, m, 3] strides (12, 1)
                    in1 = bass.AP(gj.tensor, gj.offset, [gj.ap[0], gj.ap[1], [0, 4], gj.ap[2]])
                    out_ap = pj[:, :].rearrange("p (m i c) -> p m i c", i=4, c=3)
                    nc.vector.tensor_tensor(out=out_ap, in0=in0, in1=in1, op=mybir.AluOpType.mult)
                    p_tiles.append(pj)
                s01 = work_pool.tile([P, 12 * m], F32, tag="s01")
                s23 = work_pool.tile([P, 12 * m], F32, tag="s23")
                nc.vector.tensor_tensor(out=s01[:, :], in0=p_tiles[0][:, :], in1=p_tiles[1][:, :], op=mybir.AluOpType.add)
                nc.vector.tensor_tensor(out=s23[:, :], in0=p_tiles[2][:, :], in1=p_tiles[3][:, :], op=mybir.AluOpType.add)
                nc.vector.tensor_tensor(out=out_t[:, :], in0=s01[:, :], in1=s23[:, :], op=mybir.AluOpType.add)
                nc.sync.dma_start(
                    out=out[e0:e1, :, :].rearrange("(p m) i c -> p (m i c)", p=P),
                    in_=out_t[:, :],
                )
    nc.compile()
    return nc


def kernel(convection_vectors, mesh_elements, inv_matrices):
    global LAST_RESULTS
    table = np.ascontiguousarray(convection_vectors, dtype=np.float32)
    mesh = np.ascontiguousarray(mesh_elements, dtype=np.int32)
    inv = np.ascontiguousarray(inv_matrices, dtype=np.float32)
    e_total = mesh.shape[0]
    assert e_total == N_ELEMENTS and table.shape[0] == N_NODES

    if "nc" not in _CACHE:
        _CACHE["nc"] = _build_nc()
    nc = _CACHE["nc"]

    pad = E_PAD - E_CORE
    in_maps = []
    for c in range(N_CORES):
        lo = c * E_CORE
        hi = lo + E_CORE
        mesh_c = np.concatenate([mesh[lo:hi], np.zeros((pad, 4), np.int32)], axis=0)
        inv_c = np.concatenate([inv[lo:hi], np.zeros((pad, 4, 4), np.float32)], axis=0)
        in_maps.append(
            {
                "convection_vectors": table,
                "mesh_elements": mesh_c,
                "inv_matrices": inv_c,
            }
        )

    trace = bool(os.environ.get("BASS_KERNEL_TRACE"))
    if trace:
        _ensure_ntff_hook()
    res = run_bass_kernel_spmd(nc, in_maps, core_ids=list(range(N_CORES)), trace=trace)
    LAST_RESULTS = res
    out = np.concatenate([r["out"][:E_CORE] for r in res.results], axis=0)
    return out.astype(np.float32, copy=False)
